# revision 16
# baseline (speedup 1.0000x reference)
"""Trainium2 Bass kernel for nn_MLPBuilder (GNN message-passing edge predictor).

Math: adj[i,j] = argmax_o softmax(W2 @ relu(W1 @ cat(x_i, x_j) + b1) + b2)
            = 1  iff  w . relu(la_i + lb_j + b1) + c > 0
  where la = x @ W1[:, :D].T, lb = x @ W1[:, D:].T,
        w = W2[1] - W2[0], c = b2[1] - b2[0]   (softmax+argmax == threshold).

Sharding: rows of the N^2 pair grid, 128 i-rows per core (8 cores).

Per core (setup fp32; pair-grid phase in fp16 - measured per-engine rates):
 - lbT[hh][h', j]  [128, 1024]: lb transposed, h on partitions (hh = h-half)
 - labT[hh][h', i] [128, 128] : la + b1 transposed (per-partition relu bias)
 - relu tiles (fp16 out everywhere; engines round-to-nearest like np.float16):
     ScalarE activation(Relu, bias)    h-half 0, j in [0, 512)    (1.0 ns/col)
     VectorE tensor_scalar(add,max)    h-half 0 j in [512, 1024) and all of
             h-half 1, reading an fp16 copy of lbT (0.39 ns/col fp16-in)
   GpSimd tensor_scalar measures ~8us/op on HW - not used.
 - h-reduction on PE, fp16 (1 cycle/row): stationary is a [128, 128] sliding
   view into b16[hh] [128, 320] holding fp16-hi(w) at col 127 and fp16-lo
   (w - hi) at col 191; view offset 127-c places hi at column c, lo at c+64,
   so psum row c accumulates left-node i's hi logit and row 64+c the lo
   correction in the SAME 4 matmuls (4 x 512 rows = 2048 rows/i).
   64 i-rows per psum group x 2 j-halves x 2 groups = 4 banks.
 - evacuation per bank (engines cannot cross partitions; DMA can):
   VectorE copy psum->SBUF, DMA rows [64:128) down to a [64,512] tile,
   VectorE add hi+lo, ScalarE Sign(sum + c) -> uint8, DMA out 64 rows.
   Group-0 evacuation overlaps group-1 compute.

Numerics: simulated end-to-end (np.float16 rounding at each engine write,
fp32 psum accumulation) flips 84 of 2^20 entries -> rel err 0.0164 < 2e-2.
"""

import numpy as np

import concourse.bass as bass
import concourse.bacc as bacc
import concourse.mybir as mybir
from concourse.tile import TileContext
from concourse.bass_utils import run_bass_kernel_spmd

N, D, H = 1024, 128, 256
NCORES = 8
RPC = N // NCORES  # 128 i-rows per core
GRP = 64  # i-rows per psum group
FP32 = mybir.dt.float32
FP16 = mybir.dt.float16

# inA columns: [w1bT (256) | xT (1024)]
A_W1B, A_XT = 0, 256
# inB columns: [w1aT (256) | xiT (128) | b16h0 (320) | b16h1 (320) |
#               spair (64) | b1c (2)]
B_W1A, B_XI, B_B16H0, B_B16H1, B_SP, B_B1C = 0, 256, 384, 704, 1024, 1088
NB = 1090

TRACE = False
LAST_RESULTS = None


def build_nc(cdiff: float):
    AF = mybir.ActivationFunctionType
    ALU = mybir.AluOpType

    nc = bacc.Bacc(None, target_bir_lowering=False)
    inA = nc.declare_dram_parameter("inA", [128, 1280], FP32, isOutput=False)
    inB = nc.declare_dram_parameter("inB", [128, NB], FP32, isOutput=False)
    adj8 = nc.declare_dram_parameter("adj8", [RPC, N], mybir.dt.uint8, isOutput=True)

    with TileContext(nc) as tc:
        with (
            tc.tile_pool(name="const", bufs=1) as cpool,
            tc.tile_pool(name="relu", bufs=12) as rpool,
            tc.tile_pool(name="evac", bufs=2) as epool,
            tc.tile_pool(name="mm", bufs=2, space="PSUM") as mmpool,
            tc.tile_pool(name="setup_ps", bufs=2, space="PSUM") as spool,
            tc.tile_pool(name="setup_ps2", bufs=1, space="PSUM") as spool2,
        ):
            inA_sb = cpool.tile([128, 1280], FP32)
            # chunk 0 carries w1bT + xT[:, :512], split across two DMA
            # queues; chunk 1 the rest of xT
            nc.sync.dma_start(out=inA_sb[:, :384], in_=inA[:, :384])
            nc.sync.dma_start(out=inA_sb[:, 384:768], in_=inA[:, 384:768])
            nc.sync.dma_start(out=inA_sb[:, 768:], in_=inA[:, 768:])
            inB_sb = cpool.tile([128, NB], FP32)
            nc.sync.dma_start(out=inB_sb[:, :545], in_=inB[:, :545])
            nc.sync.dma_start(out=inB_sb[:, 545:], in_=inB[:, 545:])

            # warmup matmuls on a zeroed tile: keeps the PE busy during the
            # input DMA so the HAM clock gate ramps before the real work
            wz = cpool.tile([128, 512], FP32)
            nc.gpsimd.memset(wz[:], 0.0)
            for _ in range(2):
                wps = spool.tile([128, 512], FP32, tag="setup_ps", name="wps")
                nc.tensor.matmul(
                    wps[:], wz[:, 0:128], wz[:], start=True, stop=True
                )

            w1bT_sb = inA_sb[:, A_W1B : A_W1B + 256]
            xT_sb = inA_sb[:, A_XT : A_XT + 1024]
            w1aT_sb = inB_sb[:, B_W1A : B_W1A + 256]
            xiT_sb = inB_sb[:, B_XI : B_XI + 128]
            b1c_sb = inB_sb[:, B_B1C : B_B1C + 2]

            # cbias: [128,1] = cdiff, for the Sign evacuation
            cbias = cpool.tile([128, 1], FP32)
            nc.vector.memset(cbias[:], cdiff)
            # ScalarE pre-touch of inB so later ACT ops never add a DMA wait
            sct = cpool.tile([128, 1], FP32)
            nc.scalar.copy(sct[:], inB_sb[:, B_B1C : B_B1C + 1])

            # stationaries: b16[hh] [128, 320] fp16, w-hi at col 127 and
            # w-lo at col 191 (cast on-chip: matmul operands must come from
            # a rounding engine op, not DMA)
            b16 = []
            for hh, off in ((0, B_B16H0), (1, B_B16H1)):
                t = cpool.tile([128, 320], FP16, tag=f"b16_{hh}", name=f"b16_{hh}")
                nc.vector.tensor_copy(t[:], inB_sb[:, off : off + 320])
                b16.append(t)


            # ---- lbT[hh] = (x @ W1b.T).T, h on partitions ----
            lbT = []
            for hh in range(2):
                t = cpool.tile([128, N], FP32, tag=f"lbT{hh}", name=f"lbT{hh}")
                lbT.append(t)
            for jc in range(2):  # jc outer: chunk-0 DMA gates jc=0 MMs only
                if jc == 1:
                    # wait-collector: absorb the chunk-1 DMA wait on PE so the
                    # real jc=1 matmuls carry only their PSUM-WAR wait
                    dps = spool.tile([128, 512], FP32, tag="setup_ps", name="dps")
                    nc.tensor.matmul(
                        dps[0:1, 0:1],
                        w1bT_sb[:, 0:1],
                        xT_sb[:, 1023:1024],
                        start=True,
                        stop=True,
                    )
                for hh in range(2):
                    ps = spool.tile([128, 512], FP32, tag="setup_ps", name="ps_lb")
                    nc.tensor.matmul(
                        ps[:],
                        w1bT_sb[:, hh * 128 : (hh + 1) * 128],
                        xT_sb[:, jc * 512 : (jc + 1) * 512],
                        start=True,
                        stop=True,
                    )
                    if jc == 0:
                        nc.vector.tensor_copy(
                            lbT[hh][:, jc * 512 : (jc + 1) * 512], ps[:]
                        )
                    else:
                        nc.scalar.copy(lbT[hh][:, jc * 512 : (jc + 1) * 512], ps[:])

            # fp16 copies of the VectorE-owned lbT slices (fp16 input unlocks
            # the DVE 2-byte fast path)
            lb16_h1 = cpool.tile([128, N], FP16, tag="lb16h1", name="lb16h1")
            nc.vector.tensor_copy(lb16_h1[:], lbT[1][:])
            lb16_h0t = cpool.tile([128, 512], FP16, tag="lb16h0t", name="lb16h0t")
            nc.vector.tensor_copy(lb16_h0t[:], lbT[0][:, 512:1024])

            # ---- labT[hh] = (x_i @ W1a.T).T + b1, h on partitions ----
            labT = []
            for hh in range(2):
                t = cpool.tile([128, RPC], FP32, tag=f"labT{hh}", name=f"labT{hh}")
                labT.append(t)
                ps = spool2.tile([128, RPC], FP32, tag="setup_ps2", name="ps_la")
                nc.tensor.matmul(
                    ps[:],
                    w1aT_sb[:, hh * 128 : (hh + 1) * 128],
                    xiT_sb[:],
                    start=True,
                    stop=True,
                )
                nc.scalar.activation(
                    t[:], ps[:], AF.Identity, bias=b1c_sb[:, hh : hh + 1], scale=1.0
                )

            # ---- main loop: psum row i%64 (hi) and 64+i%64 (lo) per i ----
            DEFER = 8  # trailing i-rows whose psB matmuls run after psA closes
            for g in range(2):
                psA = mmpool.tile([128, 512], FP32, tag="mmA", name="psA")
                psB = mmpool.tile([128, 512], FP32, tag="mmB", name="psB")
                pend = []  # deferred psB matmuls for the group tail
                for c in range(GRP):
                    i = g * GRP + c
                    first = c == 0
                    last = c == GRP - 1
                    defer = g == 1 and c >= GRP - DEFER
                    st0 = b16[0][:, 127 - c : 255 - c]
                    st1 = b16[1][:, 127 - c : 255 - c]

                    tA = rpool.tile([128, 512], FP16, tag="tA", name="tA")
                    tV0 = rpool.tile([128, 512], FP16, tag="tV0", name="tV0")
                    tV1 = rpool.tile([128, 1024], FP16, tag="tV1", name="tV1")
                    nc.scalar.activation(
                        tA[:],
                        lbT[0][:, 0:512],
                        AF.Relu,
                        bias=labT[0][:, i : i + 1],
                        scale=1.0,
                    )
                    nc.vector.tensor_scalar(
                        tV0[:], lb16_h0t[:], labT[0][:, i : i + 1],
                        0.0, ALU.add, ALU.max,
                    )
                    nc.vector.tensor_scalar(
                        tV1[:], lb16_h1[:], labT[1][:, i : i + 1],
                        0.0, ALU.add, ALU.max,
                    )
                    nc.tensor.matmul(psA[:], st0, tA[:], start=first, stop=False)
                    nc.tensor.matmul(
                        psA[:], st1, tV1[:, 0:512], start=False, stop=last
                    )
                    if defer:
                        pend.append((st0, tV0, st1, tV1, last))
                    else:
                        nc.tensor.matmul(
                            psB[:], st0, tV0[:], start=first, stop=False
                        )
                        nc.tensor.matmul(
                            psB[:], st1, tV1[:, 512:1024], start=False, stop=last
                        )

                def evac(jc, ps):
                    # logit row c = psum row c + psum row 64+c; engines cannot
                    # cross partitions, so bounce the lo rows through a DMA
                    full = epool.tile([128, 512], FP32, tag=f"fl{jc}", name="fl")
                    nc.vector.tensor_copy(full[:], ps[:])
                    shf = epool.tile([64, 512], FP32, tag=f"sh{jc}", name="sh")
                    nc.sync.dma_start(out=shf[:], in_=full[64:128, :])
                    osum = epool.tile([64, 512], FP32, tag=f"os{jc}", name="os")
                    nc.vector.tensor_tensor(
                        osum[:], full[0:64, :], shf[:], ALU.add
                    )
                    at = epool.tile(
                        [64, 512], mybir.dt.uint8, tag=f"adjt{jc}", name="at"
                    )
                    nc.scalar.activation(
                        at[:], osum[:], AF.Sign, bias=cbias[0:64, :], scale=1.0
                    )
                    nc.sync.dma_start(
                        out=adj8[g * GRP : (g + 1) * GRP, jc * 512 : (jc + 1) * 512],
                        in_=at[:],
                    )

                # psA is fully accumulated; its evacuation overlaps the
                # deferred psB matmuls
                evac(0, psA)
                for st0, tV0, st1, tV1, last in pend:
                    nc.tensor.matmul(psB[:], st0, tV0[:], start=False, stop=last)
                    nc.tensor.matmul(
                        psB[:], st1, tV1[:, 512:1024], start=False, stop=last
                    )
                evac(1, psB)
    nc.compile()
    return nc


def _prep_inputs(x, W1, b1, W2, b2):
    x = np.asarray(x, dtype=np.float32)
    W1 = np.asarray(W1, dtype=np.float32)
    b1 = np.asarray(b1, dtype=np.float32)
    W2 = np.asarray(W2, dtype=np.float32)
    b2 = np.asarray(b2, dtype=np.float32)

    xT = np.ascontiguousarray(x.T)  # [D, N]
    w1aT = np.ascontiguousarray(W1[:, :D].T)  # [D, H]
    w1bT = np.ascontiguousarray(W1[:, D:].T)  # [D, H]
    b1c = np.ascontiguousarray(b1.reshape(2, 128).T)  # [128, 2]
    w = (W2[1] - W2[0]).astype(np.float32)  # [H]
    cdiff = float(np.float32(b2[1]) - np.float32(b2[0]))

    b16 = np.zeros((128, 2, 320), dtype=np.float32)
    for hh in range(2):
        whh = w[hh * 128 : (hh + 1) * 128]
        hi = whh.astype(np.float16).astype(np.float32)
        lo = (whh - hi).astype(np.float16).astype(np.float32)
        b16[:, hh, 127] = hi
        b16[:, hh, 191] = lo
    spair = np.zeros((128, 64), dtype=np.float32)
    for c in range(64):
        spair[c, c] = 1.0
        spair[64 + c, c] = 1.0
    inA = np.concatenate([w1bT, xT], axis=1)  # [128, 1280]
    return xT, w1aT, b1c, b16, spair, inA, cdiff


def kernel(x, W1, b1, W2, b2):
    global LAST_RESULTS
    xT, w1aT, b1c, b16, spair, inA, cdiff = _prep_inputs(x, W1, b1, W2, b2)

    nc = build_nc(cdiff)
    in_maps = []
    for core in range(NCORES):
        xiT = xT[:, core * RPC : (core + 1) * RPC]
        inB = np.concatenate(
            [w1aT, xiT, b16.reshape(128, 640), spair, b1c], axis=1
        )  # [128, 1090]
        in_maps.append(dict(inA=inA, inB=np.ascontiguousarray(inB)))
    res = run_bass_kernel_spmd(nc, in_maps, list(range(NCORES)), trace=TRACE)
    LAST_RESULTS = res
    adj = np.concatenate(
        [(res.results[c]["adj8"] == 1) for c in range(NCORES)], axis=0
    ).astype(np.int32)
    np.fill_diagonal(adj, 1)
    return adj


# revision 18
# speedup vs baseline: 1.1658x; 1.1658x over previous
"""Trainium2 Bass kernel for nn_MLPBuilder (GNN message-passing edge predictor).

Math: adj[i,j] = argmax_o softmax(W2 @ relu(W1 @ cat(x_i, x_j) + b1) + b2)
            = 1  iff  w . relu(la_i + lb_j + b1) + c > 0
  where la = x @ W1[:, :D].T, lb = x @ W1[:, D:].T,
        w = W2[1] - W2[0], c = b2[1] - b2[0]   (softmax+argmax == threshold).

Sharding: rows of the N^2 pair grid, 128 i-rows per core (8 cores).

Per core (setup fp32; pair-grid phase in fp16 - measured per-engine rates):
 - lbT[hh][h', j]  [128, 1024]: lb transposed, h on partitions (hh = h-half)
 - labT[hh][h', i] [128, 128] : la + b1 transposed (per-partition relu bias)
 - relu tiles (fp16 out everywhere; engines round-to-nearest like np.float16):
     ScalarE activation(Relu, bias)    h-half 0, j in [0, 512)    (1.0 ns/col)
     VectorE tensor_scalar(add,max)    h-half 0 j in [512, 1024) and all of
             h-half 1, reading an fp16 copy of lbT (0.39 ns/col fp16-in)
   GpSimd tensor_scalar measures ~8us/op on HW - not used.
 - h-reduction on PE, fp16 (1 cycle/row): stationary is a [128, 128] sliding
   view into b16[hh] [128, 320] holding fp16-hi(w) at col 127 and fp16-lo
   (w - hi) at col 191; view offset 127-c places hi at column c, lo at c+64,
   so psum row c accumulates left-node i's hi logit and row 64+c the lo
   correction in the SAME 4 matmuls (4 x 512 rows = 2048 rows/i).
   64 i-rows per psum group x 2 j-halves x 2 groups = 4 banks.
 - evacuation per bank (engines cannot cross partitions; DMA can):
   VectorE copy psum->SBUF, DMA rows [64:128) down to a [64,512] tile,
   VectorE add hi+lo, ScalarE Sign(sum + c) -> uint8, DMA out 64 rows.
   Group-0 evacuation overlaps group-1 compute.

Numerics: simulated end-to-end (np.float16 rounding at each engine write,
fp32 psum accumulation) flips 84 of 2^20 entries -> rel err 0.0164 < 2e-2.
"""

import numpy as np

import concourse.bass as bass
import concourse.bacc as bacc
import concourse.mybir as mybir
from concourse.tile import TileContext
from concourse.bass_utils import run_bass_kernel_spmd

N, D, H = 1024, 128, 256
NCORES = 8
RPC = N // NCORES  # 128 i-rows per core
GRP = 64  # i-rows per psum group
FP32 = mybir.dt.float32
FP16 = mybir.dt.float16

# inA columns: [w1bT (256) | xT (1024)]
A_W1B, A_XT = 0, 256
# inB columns: [w1aT (256) | xiT (128) | b16h0 (320) | b16h1 (320) |
#               spair (64) | b1c (2)]
B_W1A, B_XI, B_B16H0, B_B16H1, B_SP, B_B1C = 0, 256, 384, 704, 1024, 1088
NB = 1090

TRACE = False
LAST_RESULTS = None


def build_nc(cdiff: float):
    AF = mybir.ActivationFunctionType
    ALU = mybir.AluOpType

    nc = bacc.Bacc(None, target_bir_lowering=False)
    inA = nc.declare_dram_parameter("inA", [128, 1280], FP32, isOutput=False)
    inB = nc.declare_dram_parameter("inB", [128, NB], FP32, isOutput=False)
    adj8 = nc.declare_dram_parameter("adj8", [RPC, N], mybir.dt.uint8, isOutput=True)

    with TileContext(nc) as tc:
        with (
            tc.tile_pool(name="const", bufs=1) as cpool,
            tc.tile_pool(name="relu", bufs=3) as rpool,
            tc.tile_pool(name="evac", bufs=2) as epool,
            tc.tile_pool(name="mm", bufs=2, space="PSUM") as mmpool,
            tc.tile_pool(name="setup_ps", bufs=2, space="PSUM") as spool,
            tc.tile_pool(name="setup_ps2", bufs=1, space="PSUM") as spool2,
        ):
            inA_sb = cpool.tile([128, 1280], FP32)
            # chunk 0 carries w1bT + xT[:, :512], split across two DMA
            # queues; chunk 1 the rest of xT
            nc.sync.dma_start(out=inA_sb[:, :384], in_=inA[:, :384])
            nc.sync.dma_start(out=inA_sb[:, 384:768], in_=inA[:, 384:768])
            nc.sync.dma_start(out=inA_sb[:, 768:], in_=inA[:, 768:])
            inB_sb = cpool.tile([128, NB], FP32)
            nc.sync.dma_start(out=inB_sb[:, :545], in_=inB[:, :545])
            nc.sync.dma_start(out=inB_sb[:, 545:], in_=inB[:, 545:])

            # warmup matmuls on a zeroed tile: keeps the PE busy during the
            # input DMA so the HAM clock gate ramps before the real work
            wz = cpool.tile([128, 512], FP32)
            nc.gpsimd.memset(wz[:], 0.0)
            for _ in range(2):
                wps = spool.tile([128, 512], FP32, tag="setup_ps", name="wps")
                nc.tensor.matmul(
                    wps[:], wz[:, 0:128], wz[:], start=True, stop=True
                )

            w1bT_sb = inA_sb[:, A_W1B : A_W1B + 256]
            xT_sb = inA_sb[:, A_XT : A_XT + 1024]
            w1aT_sb = inB_sb[:, B_W1A : B_W1A + 256]
            xiT_sb = inB_sb[:, B_XI : B_XI + 128]
            b1c_sb = inB_sb[:, B_B1C : B_B1C + 2]

            # cbias: [128,1] = cdiff, for the Sign evacuation
            cbias = cpool.tile([128, 1], FP32)
            nc.vector.memset(cbias[:], cdiff)
            # ScalarE pre-touch of inB so later ACT ops never add a DMA wait
            sct = cpool.tile([128, 1], FP32)
            nc.scalar.copy(sct[:], inB_sb[:, B_B1C : B_B1C + 1])

            # stationaries: b16[hh] [128, 320] fp16, w-hi at col 127 and
            # w-lo at col 191 (cast on-chip: matmul operands must come from
            # a rounding engine op, not DMA)
            b16 = []
            for hh, off in ((0, B_B16H0), (1, B_B16H1)):
                t = cpool.tile([128, 320], FP16, tag=f"b16_{hh}", name=f"b16_{hh}")
                nc.vector.tensor_copy(t[:], inB_sb[:, off : off + 320])
                b16.append(t)


            # ---- lbT[hh] = (x @ W1b.T).T, h on partitions ----
            lbT = []
            for hh in range(2):
                t = cpool.tile([128, N], FP32, tag=f"lbT{hh}", name=f"lbT{hh}")
                lbT.append(t)
            for jc in range(2):  # jc outer: chunk-0 DMA gates jc=0 MMs only
                if jc == 1:
                    # wait-collector: absorb the chunk-1 DMA wait on PE so the
                    # real jc=1 matmuls carry only their PSUM-WAR wait
                    dps = spool.tile([128, 512], FP32, tag="setup_ps", name="dps")
                    nc.tensor.matmul(
                        dps[0:1, 0:1],
                        w1bT_sb[:, 0:1],
                        xT_sb[:, 1023:1024],
                        start=True,
                        stop=True,
                    )
                for hh in range(2):
                    ps = spool.tile([128, 512], FP32, tag="setup_ps", name="ps_lb")
                    nc.tensor.matmul(
                        ps[:],
                        w1bT_sb[:, hh * 128 : (hh + 1) * 128],
                        xT_sb[:, jc * 512 : (jc + 1) * 512],
                        start=True,
                        stop=True,
                    )
                    if jc == 0:
                        nc.vector.tensor_copy(
                            lbT[hh][:, jc * 512 : (jc + 1) * 512], ps[:]
                        )
                    else:
                        nc.scalar.copy(lbT[hh][:, jc * 512 : (jc + 1) * 512], ps[:])

            # fp16 copies of the VectorE-owned lbT slices (fp16 input unlocks
            # the DVE 2-byte fast path)
            lb16_h1 = cpool.tile([128, N], FP16, tag="lb16h1", name="lb16h1")
            nc.vector.tensor_copy(lb16_h1[:], lbT[1][:])
            lb16_h0t = cpool.tile([128, 512], FP16, tag="lb16h0t", name="lb16h0t")
            nc.vector.tensor_copy(lb16_h0t[:], lbT[0][:, 512:1024])

            # ---- labT[hh] = (x_i @ W1a.T).T + b1, h on partitions ----
            labT = []
            for hh in range(2):
                t = cpool.tile([128, RPC], FP32, tag=f"labT{hh}", name=f"labT{hh}")
                labT.append(t)
                ps = spool2.tile([128, RPC], FP32, tag="setup_ps2", name="ps_la")
                nc.tensor.matmul(
                    ps[:],
                    w1aT_sb[:, hh * 128 : (hh + 1) * 128],
                    xiT_sb[:],
                    start=True,
                    stop=True,
                )
                nc.scalar.activation(
                    t[:], ps[:], AF.Identity, bias=b1c_sb[:, hh : hh + 1], scale=1.0
                )

            # ---- main loop: psum row i%64 (hi) and 64+i%64 (lo) per i ----
            DEFER = 0  # trailing i-rows whose psB matmuls run after psA closes
            for g in range(2):
                psA = mmpool.tile([128, 512], FP32, tag="mmA", name="psA")
                psB = mmpool.tile([128, 512], FP32, tag="mmB", name="psB")
                pend = []  # deferred psB matmuls for the group tail
                for c in range(GRP):
                    i = g * GRP + c
                    first = c == 0
                    last = c == GRP - 1
                    defer = g == 1 and c >= GRP - DEFER
                    st0 = b16[0][:, 127 - c : 255 - c]
                    st1 = b16[1][:, 127 - c : 255 - c]

                    tA = rpool.tile([128, 512], FP16, tag="tA", name="tA")
                    tV0 = rpool.tile([128, 512], FP16, tag="tV0", name="tV0")
                    tV1 = rpool.tile([128, 1024], FP16, tag="tV1", name="tV1")
                    nc.scalar.activation(
                        tA[:],
                        lbT[0][:, 0:512],
                        AF.Relu,
                        bias=labT[0][:, i : i + 1],
                        scale=1.0,
                    )
                    nc.vector.tensor_scalar(
                        tV0[:], lb16_h0t[:], labT[0][:, i : i + 1],
                        0.0, ALU.add, ALU.max,
                    )
                    nc.vector.tensor_scalar(
                        tV1[:], lb16_h1[:], labT[1][:, i : i + 1],
                        0.0, ALU.add, ALU.max,
                    )
                    nc.tensor.matmul(psA[:], st0, tA[:], start=first, stop=False)
                    nc.tensor.matmul(
                        psA[:], st1, tV1[:, 0:512], start=False, stop=last
                    )
                    if defer:
                        pend.append((st0, tV0, st1, tV1, last))
                    else:
                        nc.tensor.matmul(
                            psB[:], st0, tV0[:], start=first, stop=False
                        )
                        nc.tensor.matmul(
                            psB[:], st1, tV1[:, 512:1024], start=False, stop=last
                        )

                def evac(jc, ps):
                    # logit row c = psum row c + psum row 64+c; engines cannot
                    # cross partitions, so bounce the lo rows through a DMA
                    full = epool.tile([128, 512], FP32, tag=f"fl{jc}", name="fl")
                    nc.vector.tensor_copy(full[:], ps[:])
                    shf = epool.tile([64, 512], FP32, tag=f"sh{jc}", name="sh")
                    nc.sync.dma_start(out=shf[:], in_=full[64:128, :])
                    osum = epool.tile([64, 512], FP32, tag=f"os{jc}", name="os")
                    nc.vector.tensor_tensor(
                        osum[:], full[0:64, :], shf[:], ALU.add
                    )
                    at = epool.tile(
                        [64, 512], mybir.dt.uint8, tag=f"adjt{jc}", name="at"
                    )
                    nc.scalar.activation(
                        at[:], osum[:], AF.Sign, bias=cbias[0:64, :], scale=1.0
                    )
                    nc.sync.dma_start(
                        out=adj8[g * GRP : (g + 1) * GRP, jc * 512 : (jc + 1) * 512],
                        in_=at[:],
                    )

                # psA is fully accumulated; its evacuation overlaps the
                # deferred psB matmuls
                evac(0, psA)
                for st0, tV0, st1, tV1, last in pend:
                    nc.tensor.matmul(psB[:], st0, tV0[:], start=False, stop=last)
                    nc.tensor.matmul(
                        psB[:], st1, tV1[:, 512:1024], start=False, stop=last
                    )
                evac(1, psB)
    nc.compile()
    return nc


def _prep_inputs(x, W1, b1, W2, b2):
    x = np.asarray(x, dtype=np.float32)
    W1 = np.asarray(W1, dtype=np.float32)
    b1 = np.asarray(b1, dtype=np.float32)
    W2 = np.asarray(W2, dtype=np.float32)
    b2 = np.asarray(b2, dtype=np.float32)

    xT = np.ascontiguousarray(x.T)  # [D, N]
    w1aT = np.ascontiguousarray(W1[:, :D].T)  # [D, H]
    w1bT = np.ascontiguousarray(W1[:, D:].T)  # [D, H]
    b1c = np.ascontiguousarray(b1.reshape(2, 128).T)  # [128, 2]
    w = (W2[1] - W2[0]).astype(np.float32)  # [H]
    cdiff = float(np.float32(b2[1]) - np.float32(b2[0]))

    b16 = np.zeros((128, 2, 320), dtype=np.float32)
    for hh in range(2):
        whh = w[hh * 128 : (hh + 1) * 128]
        hi = whh.astype(np.float16).astype(np.float32)
        lo = (whh - hi).astype(np.float16).astype(np.float32)
        b16[:, hh, 127] = hi
        b16[:, hh, 191] = lo
    spair = np.zeros((128, 64), dtype=np.float32)
    for c in range(64):
        spair[c, c] = 1.0
        spair[64 + c, c] = 1.0
    inA = np.concatenate([w1bT, xT], axis=1)  # [128, 1280]
    return xT, w1aT, b1c, b16, spair, inA, cdiff


def kernel(x, W1, b1, W2, b2):
    global LAST_RESULTS
    xT, w1aT, b1c, b16, spair, inA, cdiff = _prep_inputs(x, W1, b1, W2, b2)

    nc = build_nc(cdiff)
    in_maps = []
    for core in range(NCORES):
        xiT = xT[:, core * RPC : (core + 1) * RPC]
        inB = np.concatenate(
            [w1aT, xiT, b16.reshape(128, 640), spair, b1c], axis=1
        )  # [128, 1090]
        in_maps.append(dict(inA=inA, inB=np.ascontiguousarray(inB)))
    res = run_bass_kernel_spmd(nc, in_maps, list(range(NCORES)), trace=TRACE)
    LAST_RESULTS = res
    adj = np.concatenate(
        [(res.results[c]["adj8"] == 1) for c in range(NCORES)], axis=0
    ).astype(np.int32)
    np.fill_diagonal(adj, 1)
    return adj


# revision 19
# speedup vs baseline: 1.1763x; 1.0090x over previous
"""Trainium2 Bass kernel for nn_MLPBuilder (GNN message-passing edge predictor).

Math: adj[i,j] = argmax_o softmax(W2 @ relu(W1 @ cat(x_i, x_j) + b1) + b2)
            = 1  iff  w . relu(la_i + lb_j + b1) + c > 0
  where la = x @ W1[:, :D].T, lb = x @ W1[:, D:].T,
        w = W2[1] - W2[0], c = b2[1] - b2[0]   (softmax+argmax == threshold).

Sharding: rows of the N^2 pair grid, 128 i-rows per core (8 cores).

Per core (setup fp32; pair-grid phase in fp16 - measured per-engine rates):
 - lbT[hh][h', j]  [128, 1024]: lb transposed, h on partitions (hh = h-half)
 - labT[hh][h', i] [128, 128] : la + b1 transposed (per-partition relu bias)
 - relu tiles (fp16 out everywhere; engines round-to-nearest like np.float16):
     ScalarE activation(Relu, bias)    h-half 0, j in [0, 512)    (1.0 ns/col)
     VectorE tensor_scalar(add,max)    h-half 0 j in [512, 1024) and all of
             h-half 1, reading an fp16 copy of lbT (0.39 ns/col fp16-in)
   GpSimd tensor_scalar measures ~8us/op on HW - not used.
 - h-reduction on PE, fp16 (1 cycle/row): stationary is a [128, 128] sliding
   view into b16[hh] [128, 320] holding fp16-hi(w) at col 127 and fp16-lo
   (w - hi) at col 191; view offset 127-c places hi at column c, lo at c+64,
   so psum row c accumulates left-node i's hi logit and row 64+c the lo
   correction in the SAME 4 matmuls (4 x 512 rows = 2048 rows/i).
   64 i-rows per psum group x 2 j-halves x 2 groups = 4 banks.
 - evacuation per bank (engines cannot cross partitions; DMA can):
   VectorE copy psum->SBUF, DMA rows [64:128) down to a [64,512] tile,
   VectorE add hi+lo, ScalarE Sign(sum + c) -> uint8, DMA out 64 rows.
   Group-0 evacuation overlaps group-1 compute.

Numerics: simulated end-to-end (np.float16 rounding at each engine write,
fp32 psum accumulation) flips 84 of 2^20 entries -> rel err 0.0164 < 2e-2.
"""

import numpy as np

import concourse.bass as bass
import concourse.bacc as bacc
import concourse.mybir as mybir
from concourse.tile import TileContext
from concourse.bass_utils import run_bass_kernel_spmd

N, D, H = 1024, 128, 256
NCORES = 8
RPC = N // NCORES  # 128 i-rows per core
GRP = 64  # i-rows per psum group
FP32 = mybir.dt.float32
FP16 = mybir.dt.float16

# inA columns: [w1bT (256) | xT (1024)]
A_W1B, A_XT = 0, 256
# inB columns: [w1aT (256) | xiT (128) | b16h0 (320) | b16h1 (320) |
#               spair (64) | b1c (2)]
B_W1A, B_XI, B_B16H0, B_B16H1, B_SP, B_B1C = 0, 256, 384, 704, 1024, 1088
NB = 1090

TRACE = False
LAST_RESULTS = None


def build_nc(cdiff: float):
    AF = mybir.ActivationFunctionType
    ALU = mybir.AluOpType

    nc = bacc.Bacc(None, target_bir_lowering=False)
    inA = nc.declare_dram_parameter("inA", [128, 1280], FP32, isOutput=False)
    inB = nc.declare_dram_parameter("inB", [128, NB], FP32, isOutput=False)
    adj8 = nc.declare_dram_parameter("adj8", [RPC, N], mybir.dt.uint8, isOutput=True)

    with TileContext(nc) as tc:
        with (
            tc.tile_pool(name="const", bufs=1) as cpool,
            tc.tile_pool(name="relu", bufs=3) as rpool,
            tc.tile_pool(name="evac", bufs=2) as epool,
            tc.tile_pool(name="mm", bufs=2, space="PSUM") as mmpool,
            tc.tile_pool(name="setup_ps", bufs=2, space="PSUM") as spool,
            tc.tile_pool(name="setup_ps2", bufs=1, space="PSUM") as spool2,
        ):
            inA_sb = cpool.tile([128, 1280], FP32)
            # chunk 0 carries w1bT + xT[:, :512], split across two DMA
            # queues; chunk 1 the rest of xT
            nc.sync.dma_start(out=inA_sb[:, :384], in_=inA[:, :384])
            nc.sync.dma_start(out=inA_sb[:, 384:768], in_=inA[:, 384:768])
            nc.sync.dma_start(out=inA_sb[:, 768:], in_=inA[:, 768:])
            inB_sb = cpool.tile([128, NB], FP32)
            nc.sync.dma_start(out=inB_sb[:, :545], in_=inB[:, :545])
            nc.sync.dma_start(out=inB_sb[:, 545:], in_=inB[:, 545:])



            w1bT_sb = inA_sb[:, A_W1B : A_W1B + 256]
            xT_sb = inA_sb[:, A_XT : A_XT + 1024]
            w1aT_sb = inB_sb[:, B_W1A : B_W1A + 256]
            xiT_sb = inB_sb[:, B_XI : B_XI + 128]
            b1c_sb = inB_sb[:, B_B1C : B_B1C + 2]

            # cbias: [128,1] = cdiff, for the Sign evacuation
            cbias = cpool.tile([128, 1], FP32)
            nc.vector.memset(cbias[:], cdiff)
            # ScalarE pre-touch of inB so later ACT ops never add a DMA wait
            sct = cpool.tile([128, 1], FP32)
            nc.scalar.copy(sct[:], inB_sb[:, B_B1C : B_B1C + 1])

            # stationaries: b16[hh] [128, 320] fp16, w-hi at col 127 and
            # w-lo at col 191 (cast on-chip: matmul operands must come from
            # a rounding engine op, not DMA)
            b16 = []
            for hh, off in ((0, B_B16H0), (1, B_B16H1)):
                t = cpool.tile([128, 320], FP16, tag=f"b16_{hh}", name=f"b16_{hh}")
                nc.vector.tensor_copy(t[:], inB_sb[:, off : off + 320])
                b16.append(t)


            # ---- lbT[hh] = (x @ W1b.T).T, h on partitions ----
            lbT = []
            for hh in range(2):
                t = cpool.tile([128, N], FP32, tag=f"lbT{hh}", name=f"lbT{hh}")
                lbT.append(t)
            for jc in range(2):  # jc outer: chunk-0 DMA gates jc=0 MMs only
                if jc == 1:
                    # wait-collector: absorb the chunk-1 DMA wait on PE so the
                    # real jc=1 matmuls carry only their PSUM-WAR wait
                    dps = spool.tile([128, 512], FP32, tag="setup_ps", name="dps")
                    nc.tensor.matmul(
                        dps[0:1, 0:1],
                        w1bT_sb[:, 0:1],
                        xT_sb[:, 1023:1024],
                        start=True,
                        stop=True,
                    )
                for hh in range(2):
                    ps = spool.tile([128, 512], FP32, tag="setup_ps", name="ps_lb")
                    nc.tensor.matmul(
                        ps[:],
                        w1bT_sb[:, hh * 128 : (hh + 1) * 128],
                        xT_sb[:, jc * 512 : (jc + 1) * 512],
                        start=True,
                        stop=True,
                    )
                    if jc == 0:
                        nc.vector.tensor_copy(
                            lbT[hh][:, jc * 512 : (jc + 1) * 512], ps[:]
                        )
                    else:
                        nc.scalar.copy(lbT[hh][:, jc * 512 : (jc + 1) * 512], ps[:])

            # fp16 copies of the VectorE-owned lbT slices (fp16 input unlocks
            # the DVE 2-byte fast path)
            lb16_h1 = cpool.tile([128, N], FP16, tag="lb16h1", name="lb16h1")
            nc.vector.tensor_copy(lb16_h1[:], lbT[1][:])
            lb16_h0t = cpool.tile([128, 512], FP16, tag="lb16h0t", name="lb16h0t")
            nc.vector.tensor_copy(lb16_h0t[:], lbT[0][:, 512:1024])

            # ---- labT[hh] = (x_i @ W1a.T).T + b1, h on partitions ----
            labT = []
            for hh in range(2):
                t = cpool.tile([128, RPC], FP32, tag=f"labT{hh}", name=f"labT{hh}")
                labT.append(t)
                ps = spool2.tile([128, RPC], FP32, tag="setup_ps2", name="ps_la")
                nc.tensor.matmul(
                    ps[:],
                    w1aT_sb[:, hh * 128 : (hh + 1) * 128],
                    xiT_sb[:],
                    start=True,
                    stop=True,
                )
                nc.scalar.activation(
                    t[:], ps[:], AF.Identity, bias=b1c_sb[:, hh : hh + 1], scale=1.0
                )

            # ---- main loop: psum row i%64 (hi) and 64+i%64 (lo) per i ----
            DEFER = 0  # trailing i-rows whose psB matmuls run after psA closes
            for g in range(2):
                psA = mmpool.tile([128, 512], FP32, tag="mmA", name="psA")
                psB = mmpool.tile([128, 512], FP32, tag="mmB", name="psB")
                pend = []  # deferred psB matmuls for the group tail
                for c in range(GRP):
                    i = g * GRP + c
                    first = c == 0
                    last = c == GRP - 1
                    defer = g == 1 and c >= GRP - DEFER
                    st0 = b16[0][:, 127 - c : 255 - c]
                    st1 = b16[1][:, 127 - c : 255 - c]

                    tA = rpool.tile([128, 512], FP16, tag="tA", name="tA")
                    tV0 = rpool.tile([128, 512], FP16, tag="tV0", name="tV0")
                    tV1 = rpool.tile([128, 1024], FP16, tag="tV1", name="tV1")
                    nc.scalar.activation(
                        tA[:],
                        lbT[0][:, 0:512],
                        AF.Relu,
                        bias=labT[0][:, i : i + 1],
                        scale=1.0,
                    )
                    nc.vector.tensor_scalar(
                        tV0[:], lb16_h0t[:], labT[0][:, i : i + 1],
                        0.0, ALU.add, ALU.max,
                    )
                    nc.vector.tensor_scalar(
                        tV1[:], lb16_h1[:], labT[1][:, i : i + 1],
                        0.0, ALU.add, ALU.max,
                    )
                    nc.tensor.matmul(psA[:], st0, tA[:], start=first, stop=False)
                    nc.tensor.matmul(
                        psA[:], st1, tV1[:, 0:512], start=False, stop=last
                    )
                    if defer:
                        pend.append((st0, tV0, st1, tV1, last))
                    else:
                        nc.tensor.matmul(
                            psB[:], st0, tV0[:], start=first, stop=False
                        )
                        nc.tensor.matmul(
                            psB[:], st1, tV1[:, 512:1024], start=False, stop=last
                        )

                def evac(jc, ps):
                    # logit row c = psum row c + psum row 64+c; engines cannot
                    # cross partitions, so bounce the lo rows through a DMA
                    full = epool.tile([128, 512], FP32, tag=f"fl{jc}", name="fl")
                    nc.vector.tensor_copy(full[:], ps[:])
                    shf = epool.tile([64, 512], FP32, tag=f"sh{jc}", name="sh")
                    nc.sync.dma_start(out=shf[:], in_=full[64:128, :])
                    osum = epool.tile([64, 512], FP32, tag=f"os{jc}", name="os")
                    nc.vector.tensor_tensor(
                        osum[:], full[0:64, :], shf[:], ALU.add
                    )
                    at = epool.tile(
                        [64, 512], mybir.dt.uint8, tag=f"adjt{jc}", name="at"
                    )
                    nc.scalar.activation(
                        at[:], osum[:], AF.Sign, bias=cbias[0:64, :], scale=1.0
                    )
                    nc.sync.dma_start(
                        out=adj8[g * GRP : (g + 1) * GRP, jc * 512 : (jc + 1) * 512],
                        in_=at[:],
                    )

                # psA is fully accumulated; its evacuation overlaps the
                # deferred psB matmuls
                evac(0, psA)
                for st0, tV0, st1, tV1, last in pend:
                    nc.tensor.matmul(psB[:], st0, tV0[:], start=False, stop=last)
                    nc.tensor.matmul(
                        psB[:], st1, tV1[:, 512:1024], start=False, stop=last
                    )
                evac(1, psB)
    nc.compile()
    return nc


def _prep_inputs(x, W1, b1, W2, b2):
    x = np.asarray(x, dtype=np.float32)
    W1 = np.asarray(W1, dtype=np.float32)
    b1 = np.asarray(b1, dtype=np.float32)
    W2 = np.asarray(W2, dtype=np.float32)
    b2 = np.asarray(b2, dtype=np.float32)

    xT = np.ascontiguousarray(x.T)  # [D, N]
    w1aT = np.ascontiguousarray(W1[:, :D].T)  # [D, H]
    w1bT = np.ascontiguousarray(W1[:, D:].T)  # [D, H]
    b1c = np.ascontiguousarray(b1.reshape(2, 128).T)  # [128, 2]
    w = (W2[1] - W2[0]).astype(np.float32)  # [H]
    cdiff = float(np.float32(b2[1]) - np.float32(b2[0]))

    b16 = np.zeros((128, 2, 320), dtype=np.float32)
    for hh in range(2):
        whh = w[hh * 128 : (hh + 1) * 128]
        hi = whh.astype(np.float16).astype(np.float32)
        lo = (whh - hi).astype(np.float16).astype(np.float32)
        b16[:, hh, 127] = hi
        b16[:, hh, 191] = lo
    spair = np.zeros((128, 64), dtype=np.float32)
    for c in range(64):
        spair[c, c] = 1.0
        spair[64 + c, c] = 1.0
    inA = np.concatenate([w1bT, xT], axis=1)  # [128, 1280]
    return xT, w1aT, b1c, b16, spair, inA, cdiff


def kernel(x, W1, b1, W2, b2):
    global LAST_RESULTS
    xT, w1aT, b1c, b16, spair, inA, cdiff = _prep_inputs(x, W1, b1, W2, b2)

    nc = build_nc(cdiff)
    in_maps = []
    for core in range(NCORES):
        xiT = xT[:, core * RPC : (core + 1) * RPC]
        inB = np.concatenate(
            [w1aT, xiT, b16.reshape(128, 640), spair, b1c], axis=1
        )  # [128, 1090]
        in_maps.append(dict(inA=inA, inB=np.ascontiguousarray(inB)))
    res = run_bass_kernel_spmd(nc, in_maps, list(range(NCORES)), trace=TRACE)
    LAST_RESULTS = res
    adj = np.concatenate(
        [(res.results[c]["adj8"] == 1) for c in range(NCORES)], axis=0
    ).astype(np.int32)
    np.fill_diagonal(adj, 1)
    return adj


# revision 21
# speedup vs baseline: 1.1821x; 1.0049x over previous
"""Trainium2 Bass kernel for nn_MLPBuilder (GNN message-passing edge predictor).

Math: adj[i,j] = argmax_o softmax(W2 @ relu(W1 @ cat(x_i, x_j) + b1) + b2)
            = 1  iff  w . relu(la_i + lb_j + b1) + c > 0
  where la = x @ W1[:, :D].T, lb = x @ W1[:, D:].T,
        w = W2[1] - W2[0], c = b2[1] - b2[0]   (softmax+argmax == threshold).

Sharding: rows of the N^2 pair grid, 128 i-rows per core (8 cores).

Per core (setup fp32; pair-grid phase in fp16 - measured per-engine rates):
 - lbT[hh][h', j]  [128, 1024]: lb transposed, h on partitions (hh = h-half)
 - labT[hh][h', i] [128, 128] : la + b1 transposed (per-partition relu bias)
 - relu tiles (fp16 out everywhere; engines round-to-nearest like np.float16):
     ScalarE activation(Relu, bias)    h-half 0, j in [0, 512)    (1.0 ns/col)
     VectorE tensor_scalar(add,max)    h-half 0 j in [512, 1024) and all of
             h-half 1, reading an fp16 copy of lbT (0.39 ns/col fp16-in)
   GpSimd tensor_scalar measures ~8us/op on HW - not used.
 - h-reduction on PE, fp16 (1 cycle/row): stationary is a [128, 128] sliding
   view into b16[hh] [128, 320] holding fp16-hi(w) at col 127 and fp16-lo
   (w - hi) at col 191; view offset 127-c places hi at column c, lo at c+64,
   so psum row c accumulates left-node i's hi logit and row 64+c the lo
   correction in the SAME 4 matmuls (4 x 512 rows = 2048 rows/i).
   64 i-rows per psum group x 2 j-halves x 2 groups = 4 banks.
 - evacuation per bank (engines cannot cross partitions; DMA can):
   VectorE copy psum->SBUF, DMA rows [64:128) down to a [64,512] tile,
   VectorE add hi+lo, ScalarE Sign(sum + c) -> uint8, DMA out 64 rows.
   Group-0 evacuation overlaps group-1 compute.

Numerics: simulated end-to-end (np.float16 rounding at each engine write,
fp32 psum accumulation) flips 84 of 2^20 entries -> rel err 0.0164 < 2e-2.
"""

import numpy as np

import concourse.bass as bass
import concourse.bacc as bacc
import concourse.mybir as mybir
from concourse.tile import TileContext
from concourse.bass_utils import run_bass_kernel_spmd

N, D, H = 1024, 128, 256
NCORES = 8
RPC = N // NCORES  # 128 i-rows per core
GRP = 64  # i-rows per psum group
FP32 = mybir.dt.float32
FP16 = mybir.dt.float16

# inA columns: [w1bT (256) | xT (1024)]
A_W1B, A_XT = 0, 256
# inB columns: [w1aT (256) | xiT (128) | b16h0 (320) | b16h1 (320) |
#               spair (64) | b1c (2)]
B_W1A, B_XI, B_B16H0, B_B16H1, B_SP, B_B1C = 0, 256, 384, 704, 1024, 1088
NB = 1090

TRACE = False
LAST_RESULTS = None


def build_nc(cdiff: float):
    AF = mybir.ActivationFunctionType
    ALU = mybir.AluOpType

    nc = bacc.Bacc(None, target_bir_lowering=False)
    inA = nc.declare_dram_parameter("inA", [128, 1280], FP32, isOutput=False)
    inB = nc.declare_dram_parameter("inB", [128, NB], FP32, isOutput=False)
    adj8 = nc.declare_dram_parameter("adj8", [RPC, N], mybir.dt.uint8, isOutput=True)

    with TileContext(nc) as tc:
        with (
            tc.tile_pool(name="const", bufs=1) as cpool,
            tc.tile_pool(name="relu", bufs=6) as rpool,
            tc.tile_pool(name="evac", bufs=2) as epool,
            tc.tile_pool(name="mm", bufs=2, space="PSUM") as mmpool,
            tc.tile_pool(name="setup_ps", bufs=2, space="PSUM") as spool,
            tc.tile_pool(name="setup_ps2", bufs=1, space="PSUM") as spool2,
        ):
            inA_sb = cpool.tile([128, 1280], FP32)
            # chunk 0 carries w1bT + xT[:, :512], split across two DMA
            # queues; chunk 1 the rest of xT
            nc.sync.dma_start(out=inA_sb[:, :384], in_=inA[:, :384])
            nc.sync.dma_start(out=inA_sb[:, 384:768], in_=inA[:, 384:768])
            nc.sync.dma_start(out=inA_sb[:, 768:], in_=inA[:, 768:])
            inB_sb = cpool.tile([128, NB], FP32)
            nc.sync.dma_start(out=inB_sb[:, :545], in_=inB[:, :545])
            nc.sync.dma_start(out=inB_sb[:, 545:], in_=inB[:, 545:])



            w1bT_sb = inA_sb[:, A_W1B : A_W1B + 256]
            xT_sb = inA_sb[:, A_XT : A_XT + 1024]
            w1aT_sb = inB_sb[:, B_W1A : B_W1A + 256]
            xiT_sb = inB_sb[:, B_XI : B_XI + 128]
            b1c_sb = inB_sb[:, B_B1C : B_B1C + 2]

            # cbias: [128,1] = cdiff, for the Sign evacuation
            cbias = cpool.tile([128, 1], FP32)
            nc.vector.memset(cbias[:], cdiff)
            # ScalarE pre-touch of inB so later ACT ops never add a DMA wait
            sct = cpool.tile([128, 1], FP32)
            nc.scalar.copy(sct[:], inB_sb[:, B_B1C : B_B1C + 1])

            # stationaries: b16[hh] [128, 320] fp16, w-hi at col 127 and
            # w-lo at col 191 (cast on-chip: matmul operands must come from
            # a rounding engine op, not DMA)
            b16 = []
            for hh, off in ((0, B_B16H0), (1, B_B16H1)):
                t = cpool.tile([128, 320], FP16, tag=f"b16_{hh}", name=f"b16_{hh}")
                nc.vector.tensor_copy(t[:], inB_sb[:, off : off + 320])
                b16.append(t)


            # ---- lbT[hh] = (x @ W1b.T).T, h on partitions ----
            lbT = []
            for hh in range(2):
                t = cpool.tile([128, N], FP32, tag=f"lbT{hh}", name=f"lbT{hh}")
                lbT.append(t)
            for jc in range(2):  # jc outer: chunk-0 DMA gates jc=0 MMs only
                if jc == 1:
                    # wait-collector: absorb the chunk-1 DMA wait on PE so the
                    # real jc=1 matmuls carry only their PSUM-WAR wait
                    dps = spool.tile([128, 512], FP32, tag="setup_ps", name="dps")
                    nc.tensor.matmul(
                        dps[0:1, 0:1],
                        w1bT_sb[:, 0:1],
                        xT_sb[:, 1023:1024],
                        start=True,
                        stop=True,
                    )
                for hh in range(2):
                    ps = spool.tile([128, 512], FP32, tag="setup_ps", name="ps_lb")
                    nc.tensor.matmul(
                        ps[:],
                        w1bT_sb[:, hh * 128 : (hh + 1) * 128],
                        xT_sb[:, jc * 512 : (jc + 1) * 512],
                        start=True,
                        stop=True,
                    )
                    if jc == 0:
                        nc.vector.tensor_copy(
                            lbT[hh][:, jc * 512 : (jc + 1) * 512], ps[:]
                        )
                    else:
                        nc.scalar.copy(lbT[hh][:, jc * 512 : (jc + 1) * 512], ps[:])

            # fp16 copies of the VectorE-owned lbT slices (fp16 input unlocks
            # the DVE 2-byte fast path)
            lb16_h1 = cpool.tile([128, N], FP16, tag="lb16h1", name="lb16h1")
            nc.vector.tensor_copy(lb16_h1[:], lbT[1][:])
            lb16_h0t = cpool.tile([128, 512], FP16, tag="lb16h0t", name="lb16h0t")
            nc.vector.tensor_copy(lb16_h0t[:], lbT[0][:, 512:1024])

            # ---- labT[hh] = (x_i @ W1a.T).T + b1, h on partitions ----
            labT = []
            for hh in range(2):
                t = cpool.tile([128, RPC], FP32, tag=f"labT{hh}", name=f"labT{hh}")
                labT.append(t)
                ps = spool2.tile([128, RPC], FP32, tag="setup_ps2", name="ps_la")
                nc.tensor.matmul(
                    ps[:],
                    w1aT_sb[:, hh * 128 : (hh + 1) * 128],
                    xiT_sb[:],
                    start=True,
                    stop=True,
                )
                nc.scalar.activation(
                    t[:], ps[:], AF.Identity, bias=b1c_sb[:, hh : hh + 1], scale=1.0
                )

            # ---- main loop: psum row i%64 (hi) and 64+i%64 (lo) per i ----
            DEFER = 4  # trailing i-rows whose psB matmuls run after psA closes
            for g in range(2):
                psA = mmpool.tile([128, 512], FP32, tag="mmA", name="psA")
                psB = mmpool.tile([128, 512], FP32, tag="mmB", name="psB")
                pend = []  # deferred psB matmuls for the group tail
                for c in range(GRP):
                    i = g * GRP + c
                    first = c == 0
                    last = c == GRP - 1
                    defer = g == 1 and c >= GRP - DEFER
                    st0 = b16[0][:, 127 - c : 255 - c]
                    st1 = b16[1][:, 127 - c : 255 - c]

                    tA = rpool.tile([128, 512], FP16, tag="tA", name="tA")
                    tV0 = rpool.tile([128, 512], FP16, tag="tV0", name="tV0")
                    tV1 = rpool.tile([128, 1024], FP16, tag="tV1", name="tV1")
                    nc.scalar.activation(
                        tA[:],
                        lbT[0][:, 0:512],
                        AF.Relu,
                        bias=labT[0][:, i : i + 1],
                        scale=1.0,
                    )
                    nc.vector.tensor_scalar(
                        tV0[:], lb16_h0t[:], labT[0][:, i : i + 1],
                        0.0, ALU.add, ALU.max,
                    )
                    nc.vector.tensor_scalar(
                        tV1[:], lb16_h1[:], labT[1][:, i : i + 1],
                        0.0, ALU.add, ALU.max,
                    )
                    nc.tensor.matmul(psA[:], st0, tA[:], start=first, stop=False)
                    nc.tensor.matmul(
                        psA[:], st1, tV1[:, 0:512], start=False, stop=last
                    )
                    if defer:
                        pend.append((st0, tV0, st1, tV1, last))
                    else:
                        nc.tensor.matmul(
                            psB[:], st0, tV0[:], start=first, stop=False
                        )
                        nc.tensor.matmul(
                            psB[:], st1, tV1[:, 512:1024], start=False, stop=last
                        )

                def evac(jc, ps):
                    # logit row c = psum row c + psum row 64+c; engines cannot
                    # cross partitions, so bounce the lo rows through a DMA
                    full = epool.tile([128, 512], FP32, tag=f"fl{jc}", name="fl")
                    nc.vector.tensor_copy(full[:], ps[:])
                    shf = epool.tile([64, 512], FP32, tag=f"sh{jc}", name="sh")
                    nc.sync.dma_start(out=shf[:], in_=full[64:128, :])
                    osum = epool.tile([64, 512], FP32, tag=f"os{jc}", name="os")
                    nc.vector.tensor_tensor(
                        osum[:], full[0:64, :], shf[:], ALU.add
                    )
                    at = epool.tile(
                        [64, 512], mybir.dt.uint8, tag=f"adjt{jc}", name="at"
                    )
                    nc.scalar.activation(
                        at[:], osum[:], AF.Sign, bias=cbias[0:64, :], scale=1.0
                    )
                    nc.sync.dma_start(
                        out=adj8[g * GRP : (g + 1) * GRP, jc * 512 : (jc + 1) * 512],
                        in_=at[:],
                    )

                # psA is fully accumulated; its evacuation overlaps the
                # deferred psB matmuls
                evac(0, psA)
                for st0, tV0, st1, tV1, last in pend:
                    nc.tensor.matmul(psB[:], st0, tV0[:], start=False, stop=last)
                    nc.tensor.matmul(
                        psB[:], st1, tV1[:, 512:1024], start=False, stop=last
                    )
                evac(1, psB)
    nc.compile()
    return nc


def _prep_inputs(x, W1, b1, W2, b2):
    x = np.asarray(x, dtype=np.float32)
    W1 = np.asarray(W1, dtype=np.float32)
    b1 = np.asarray(b1, dtype=np.float32)
    W2 = np.asarray(W2, dtype=np.float32)
    b2 = np.asarray(b2, dtype=np.float32)

    xT = np.ascontiguousarray(x.T)  # [D, N]
    w1aT = np.ascontiguousarray(W1[:, :D].T)  # [D, H]
    w1bT = np.ascontiguousarray(W1[:, D:].T)  # [D, H]
    b1c = np.ascontiguousarray(b1.reshape(2, 128).T)  # [128, 2]
    w = (W2[1] - W2[0]).astype(np.float32)  # [H]
    cdiff = float(np.float32(b2[1]) - np.float32(b2[0]))

    b16 = np.zeros((128, 2, 320), dtype=np.float32)
    for hh in range(2):
        whh = w[hh * 128 : (hh + 1) * 128]
        hi = whh.astype(np.float16).astype(np.float32)
        lo = (whh - hi).astype(np.float16).astype(np.float32)
        b16[:, hh, 127] = hi
        b16[:, hh, 191] = lo
    spair = np.zeros((128, 64), dtype=np.float32)
    for c in range(64):
        spair[c, c] = 1.0
        spair[64 + c, c] = 1.0
    inA = np.concatenate([w1bT, xT], axis=1)  # [128, 1280]
    return xT, w1aT, b1c, b16, spair, inA, cdiff


def kernel(x, W1, b1, W2, b2):
    global LAST_RESULTS
    xT, w1aT, b1c, b16, spair, inA, cdiff = _prep_inputs(x, W1, b1, W2, b2)

    nc = build_nc(cdiff)
    in_maps = []
    for core in range(NCORES):
        xiT = xT[:, core * RPC : (core + 1) * RPC]
        inB = np.concatenate(
            [w1aT, xiT, b16.reshape(128, 640), spair, b1c], axis=1
        )  # [128, 1090]
        in_maps.append(dict(inA=inA, inB=np.ascontiguousarray(inB)))
    res = run_bass_kernel_spmd(nc, in_maps, list(range(NCORES)), trace=TRACE)
    LAST_RESULTS = res
    adj = np.concatenate(
        [(res.results[c]["adj8"] == 1) for c in range(NCORES)], axis=0
    ).astype(np.int32)
    np.fill_diagonal(adj, 1)
    return adj


# revision 24
# speedup vs baseline: 1.1987x; 1.0140x over previous
"""Trainium2 Bass kernel for nn_MLPBuilder (GNN message-passing edge predictor).

Math: adj[i,j] = argmax_o softmax(W2 @ relu(W1 @ cat(x_i, x_j) + b1) + b2)
            = 1  iff  w . relu(la_i + lb_j + b1) + c > 0
  where la = x @ W1[:, :D].T, lb = x @ W1[:, D:].T,
        w = W2[1] - W2[0], c = b2[1] - b2[0]   (softmax+argmax == threshold).

Sharding: rows of the N^2 pair grid, 128 i-rows per core (8 cores).

Per core (setup fp32; pair-grid phase in fp16 - measured per-engine rates):
 - lbT[hh][h', j]  [128, 1024]: lb transposed, h on partitions (hh = h-half)
 - labT[hh][h', i] [128, 128] : la + b1 transposed (per-partition relu bias)
 - relu tiles (fp16 out everywhere; engines round-to-nearest like np.float16):
     ScalarE activation(Relu, bias)    h-half 0, j in [0, 512)    (1.0 ns/col)
     VectorE tensor_scalar(add,max)    h-half 0 j in [512, 1024) and all of
             h-half 1, reading an fp16 copy of lbT (0.39 ns/col fp16-in)
   GpSimd tensor_scalar measures ~8us/op on HW - not used.
 - h-reduction on PE, fp16 (1 cycle/row): stationary is a [128, 128] sliding
   view into b16[hh] [128, 320] holding fp16-hi(w) at col 127 and fp16-lo
   (w - hi) at col 191; view offset 127-c places hi at column c, lo at c+64,
   so psum row c accumulates left-node i's hi logit and row 64+c the lo
   correction in the SAME 4 matmuls (4 x 512 rows = 2048 rows/i).
   64 i-rows per psum group x 2 j-halves x 2 groups = 4 banks.
 - evacuation per bank (engines cannot cross partitions; DMA can):
   VectorE copy psum->SBUF, DMA rows [64:128) down to a [64,512] tile,
   VectorE add hi+lo, ScalarE Sign(sum + c) -> uint8, DMA out 64 rows.
   Group-0 evacuation overlaps group-1 compute.

Numerics: simulated end-to-end (np.float16 rounding at each engine write,
fp32 psum accumulation) flips 84 of 2^20 entries -> rel err 0.0164 < 2e-2.
"""

import numpy as np

import concourse.bass as bass
import concourse.bacc as bacc
import concourse.mybir as mybir
from concourse.tile import TileContext
from concourse.bass_utils import run_bass_kernel_spmd

N, D, H = 1024, 128, 256
NCORES = 8
RPC = N // NCORES  # 128 i-rows per core
GRP = 64  # i-rows per psum group
FP32 = mybir.dt.float32
FP16 = mybir.dt.float16

# inA columns: [w1bT (256) | xT (1024)]
A_W1B, A_XT = 0, 256
# inB columns: [w1aT (256) | xiT (128) | b16h0 (320) | b16h1 (320) |
#               spair (64) | b1c (2)]
B_W1A, B_XI, B_B16H0, B_B16H1, B_SP, B_B1C = 0, 256, 384, 704, 1024, 1088
NB = 1090

TRACE = False
LAST_RESULTS = None


def build_nc(cdiff: float):
    AF = mybir.ActivationFunctionType
    ALU = mybir.AluOpType

    nc = bacc.Bacc(None, target_bir_lowering=False)
    inA = nc.declare_dram_parameter("inA", [128, 1280], FP32, isOutput=False)
    inB = nc.declare_dram_parameter("inB", [128, NB], FP32, isOutput=False)
    adj8 = nc.declare_dram_parameter("adj8", [RPC, N], mybir.dt.uint8, isOutput=True)

    with TileContext(nc) as tc:
        with (
            tc.tile_pool(name="const", bufs=1) as cpool,
            tc.tile_pool(name="relu", bufs=6) as rpool,
            tc.tile_pool(name="evac", bufs=2) as epool,
            tc.tile_pool(name="mm", bufs=2, space="PSUM") as mmpool,
            tc.tile_pool(name="setup_ps", bufs=2, space="PSUM") as spool,
            tc.tile_pool(name="setup_ps2", bufs=1, space="PSUM") as spool2,
            tc.tile_pool(name="evac_ps", bufs=1, space="PSUM") as eppool,
        ):
            inA_sb = cpool.tile([128, 1280], FP32)
            # chunk 0 carries w1bT + xT[:, :512], split across two DMA
            # queues; chunk 1 the rest of xT
            nc.sync.dma_start(out=inA_sb[:, :384], in_=inA[:, :384])
            nc.sync.dma_start(out=inA_sb[:, 384:768], in_=inA[:, 384:768])
            nc.sync.dma_start(out=inA_sb[:, 768:], in_=inA[:, 768:])
            inB_sb = cpool.tile([128, NB], FP32)
            nc.sync.dma_start(out=inB_sb[:, :545], in_=inB[:, :545])
            nc.sync.dma_start(out=inB_sb[:, 545:], in_=inB[:, 545:])



            w1bT_sb = inA_sb[:, A_W1B : A_W1B + 256]
            xT_sb = inA_sb[:, A_XT : A_XT + 1024]
            w1aT_sb = inB_sb[:, B_W1A : B_W1A + 256]
            xiT_sb = inB_sb[:, B_XI : B_XI + 128]
            b1c_sb = inB_sb[:, B_B1C : B_B1C + 2]

            # cbias: [128,1] = cdiff, for the Sign evacuation
            cbias = cpool.tile([128, 1], FP32)
            nc.vector.memset(cbias[:], cdiff)
            # ScalarE pre-touch of inB so later ACT ops never add a DMA wait
            sct = cpool.tile([128, 1], FP32)
            nc.scalar.copy(sct[:], inB_sb[:, B_B1C : B_B1C + 1])

            # stationaries: b16[hh] [128, 320] fp16, w-hi at col 127 and
            # w-lo at col 191 (cast on-chip: matmul operands must come from
            # a rounding engine op, not DMA)
            b16 = []
            for hh, off in ((0, B_B16H0), (1, B_B16H1)):
                t = cpool.tile([128, 320], FP16, tag=f"b16_{hh}", name=f"b16_{hh}")
                nc.vector.tensor_copy(t[:], inB_sb[:, off : off + 320])
                b16.append(t)


            # ---- lbT[hh] = (x @ W1b.T).T, h on partitions ----
            lbT = []
            for hh in range(2):
                t = cpool.tile([128, N], FP32, tag=f"lbT{hh}", name=f"lbT{hh}")
                lbT.append(t)
            for jc in range(2):  # jc outer: chunk-0 DMA gates jc=0 MMs only
                if jc == 1:
                    # wait-collector: absorb the chunk-1 DMA wait on PE so the
                    # real jc=1 matmuls carry only their PSUM-WAR wait
                    dps = spool.tile([128, 512], FP32, tag="setup_ps", name="dps")
                    nc.tensor.matmul(
                        dps[0:1, 0:1],
                        w1bT_sb[:, 0:1],
                        xT_sb[:, 1023:1024],
                        start=True,
                        stop=True,
                    )
                for hh in range(2):
                    ps = spool.tile([128, 512], FP32, tag="setup_ps", name="ps_lb")
                    nc.tensor.matmul(
                        ps[:],
                        w1bT_sb[:, hh * 128 : (hh + 1) * 128],
                        xT_sb[:, jc * 512 : (jc + 1) * 512],
                        start=True,
                        stop=True,
                    )
                    if jc == 0:
                        nc.vector.tensor_copy(
                            lbT[hh][:, jc * 512 : (jc + 1) * 512], ps[:]
                        )
                    else:
                        nc.scalar.copy(lbT[hh][:, jc * 512 : (jc + 1) * 512], ps[:])

            # fp16 copies of the VectorE-owned lbT slices (fp16 input unlocks
            # the DVE 2-byte fast path)
            lb16_h1 = cpool.tile([128, N], FP16, tag="lb16h1", name="lb16h1")
            nc.vector.tensor_copy(lb16_h1[:], lbT[1][:])
            lb16_h0t = cpool.tile([128, 512], FP16, tag="lb16h0t", name="lb16h0t")
            nc.vector.tensor_copy(lb16_h0t[:], lbT[0][:, 512:1024])

            # ---- labT[hh] = (x_i @ W1a.T).T + b1, h on partitions ----
            labT = []
            for hh in range(2):
                t = cpool.tile([128, RPC], FP32, tag=f"labT{hh}", name=f"labT{hh}")
                labT.append(t)
                ps = spool2.tile([128, RPC], FP32, tag="setup_ps2", name="ps_la")
                nc.tensor.matmul(
                    ps[:],
                    w1aT_sb[:, hh * 128 : (hh + 1) * 128],
                    xiT_sb[:],
                    start=True,
                    stop=True,
                )
                nc.scalar.activation(
                    t[:], ps[:], AF.Identity, bias=b1c_sb[:, hh : hh + 1], scale=1.0
                )

            # ---- main loop: psum row i%64 (hi) and 64+i%64 (lo) per i ----
            DEFER = 4  # trailing i-rows whose psB matmuls run after psA closes
            for g in range(2):
                psA = mmpool.tile([128, 512], FP32, tag="mmA", name="psA")
                psB = mmpool.tile([128, 512], FP32, tag="mmB", name="psB")
                pend = []  # deferred psB matmuls for the group tail
                for c in range(GRP):
                    i = g * GRP + c
                    first = c == 0
                    last = c == GRP - 1
                    defer = g == 1 and c >= GRP - DEFER
                    st0 = b16[0][:, 127 - c : 255 - c]
                    st1 = b16[1][:, 127 - c : 255 - c]

                    tA = rpool.tile([128, 512], FP16, tag="tA", name="tA")
                    tV0 = rpool.tile([128, 512], FP16, tag="tV0", name="tV0")
                    tV1 = rpool.tile([128, 1024], FP16, tag="tV1", name="tV1")
                    nc.scalar.activation(
                        tA[:],
                        lbT[0][:, 0:512],
                        AF.Relu,
                        bias=labT[0][:, i : i + 1],
                        scale=1.0,
                    )
                    nc.vector.tensor_scalar(
                        tV0[:], lb16_h0t[:], labT[0][:, i : i + 1],
                        0.0, ALU.add, ALU.max,
                    )
                    nc.vector.tensor_scalar(
                        tV1[:], lb16_h1[:], labT[1][:, i : i + 1],
                        0.0, ALU.add, ALU.max,
                    )
                    nc.tensor.matmul(psA[:], st0, tA[:], start=first, stop=False)
                    nc.tensor.matmul(
                        psA[:], st1, tV1[:, 0:512], start=False, stop=last
                    )
                    if defer:
                        pend.append((st0, tV0, st1, tV1, last))
                    else:
                        nc.tensor.matmul(
                            psB[:], st0, tV0[:], start=first, stop=False
                        )
                        nc.tensor.matmul(
                            psB[:], st1, tV1[:, 512:1024], start=False, stop=last
                        )

                def evac(jc, ps, on_pe=False):
                    # logit row c = psum row c + psum row 64+c; engines cannot
                    # cross partitions, so bounce the lo rows through a DMA --
                    # except for the very last bank, where the stream is over
                    # and an fp32 PE pair-sum matmul beats the DMA latency
                    full = epool.tile([128, 512], FP32, tag=f"fl{jc}", name="fl")
                    nc.vector.tensor_copy(full[:], ps[:])
                    if on_pe:
                        spair = inB_sb[:, B_SP : B_SP + 64]
                        pse = eppool.tile([64, 512], FP32, tag="evps", name="pse")
                        nc.tensor.matmul(
                            pse[:], spair, full[:], start=True, stop=True
                        )
                        src = pse
                    else:
                        shf = epool.tile([64, 512], FP32, tag=f"sh{jc}", name="sh")
                        nc.sync.dma_start(out=shf[:], in_=full[64:128, :])
                        osum = epool.tile([64, 512], FP32, tag=f"os{jc}", name="os")
                        nc.vector.tensor_tensor(
                            osum[:], full[0:64, :], shf[:], ALU.add
                        )
                        src = osum
                    at = epool.tile(
                        [64, 512], mybir.dt.uint8, tag=f"adjt{jc}", name="at"
                    )
                    nc.scalar.activation(
                        at[:], src[:], AF.Sign, bias=cbias[0:64, :], scale=1.0
                    )
                    nc.sync.dma_start(
                        out=adj8[g * GRP : (g + 1) * GRP, jc * 512 : (jc + 1) * 512],
                        in_=at[:],
                    )

                # psA is fully accumulated; its evacuation overlaps the
                # deferred psB matmuls
                evac(0, psA)
                for st0, tV0, st1, tV1, last in pend:
                    nc.tensor.matmul(psB[:], st0, tV0[:], start=False, stop=last)
                    nc.tensor.matmul(
                        psB[:], st1, tV1[:, 512:1024], start=False, stop=last
                    )
                evac(1, psB, on_pe=(g == 1))
    nc.compile()
    return nc


def _prep_inputs(x, W1, b1, W2, b2):
    x = np.asarray(x, dtype=np.float32)
    W1 = np.asarray(W1, dtype=np.float32)
    b1 = np.asarray(b1, dtype=np.float32)
    W2 = np.asarray(W2, dtype=np.float32)
    b2 = np.asarray(b2, dtype=np.float32)

    xT = np.ascontiguousarray(x.T)  # [D, N]
    w1aT = np.ascontiguousarray(W1[:, :D].T)  # [D, H]
    w1bT = np.ascontiguousarray(W1[:, D:].T)  # [D, H]
    b1c = np.ascontiguousarray(b1.reshape(2, 128).T)  # [128, 2]
    w = (W2[1] - W2[0]).astype(np.float32)  # [H]
    cdiff = float(np.float32(b2[1]) - np.float32(b2[0]))

    b16 = np.zeros((128, 2, 320), dtype=np.float32)
    for hh in range(2):
        whh = w[hh * 128 : (hh + 1) * 128]
        hi = whh.astype(np.float16).astype(np.float32)
        lo = (whh - hi).astype(np.float16).astype(np.float32)
        b16[:, hh, 127] = hi
        b16[:, hh, 191] = lo
    spair = np.zeros((128, 64), dtype=np.float32)
    for c in range(64):
        spair[c, c] = 1.0
        spair[64 + c, c] = 1.0
    inA = np.concatenate([w1bT, xT], axis=1)  # [128, 1280]
    return xT, w1aT, b1c, b16, spair, inA, cdiff


def kernel(x, W1, b1, W2, b2):
    global LAST_RESULTS
    xT, w1aT, b1c, b16, spair, inA, cdiff = _prep_inputs(x, W1, b1, W2, b2)

    nc = build_nc(cdiff)
    in_maps = []
    for core in range(NCORES):
        xiT = xT[:, core * RPC : (core + 1) * RPC]
        inB = np.concatenate(
            [w1aT, xiT, b16.reshape(128, 640), spair, b1c], axis=1
        )  # [128, 1090]
        in_maps.append(dict(inA=inA, inB=np.ascontiguousarray(inB)))
    res = run_bass_kernel_spmd(nc, in_maps, list(range(NCORES)), trace=TRACE)
    LAST_RESULTS = res
    adj = np.concatenate(
        [(res.results[c]["adj8"] == 1) for c in range(NCORES)], axis=0
    ).astype(np.int32)
    np.fill_diagonal(adj, 1)
    return adj
